# revision 30
# baseline (speedup 1.0000x reference)
"""nn_BatchFFTMA: 9216 independent 65x65 FFT-MA simulations on 8 NeuronCores.

Math (derived from the reference):
  Only the center pixel of each normalized ifft2 is needed.  With
  rs = fftshift(R) = roll(Rc, -1) (odd N=65), Rf = Rfc * exp(i*phi),
  phi_k = 2*pi*((k1+k2) mod 65)/65 and Rfc = fft2(centered R) real/even.
  G = sqrt(Rf) = sqrt(|Rfc|) * exp(i*psi_k/2) where psi depends only on
  sign(Rfc) and k -> fixed per-k constant maps.  Per window:
    sum(v^2)*N^2 = sum_k E_k * M_k,  E = |W~|^2,  M = relu(Rfc) + relu(-Rfc)*aneg
    v_center*N^2 = sum_k [h*cpos + g*cneg]*Re(W~) + [h*spos + g*sneg]*(-Im(W~))
                   with h = sqrt(relu(Rfc)), g = sqrt(relu(-Rfc))
    mean = sum(w) * sqrt(Rfc_00) / N^2
  W~ = DFT with center phase baked into the matrices.  Rfc is reconstructed
  on-device from a theta-Fourier basis (analytic in theta, 20 harmonics).

Sharding: window/batch axis, 12 of 96 noise-row-groups per core (1152
windows each), per the data-parallel hint. Windows are ordered b = 12*c + r
per core (c = column 0..95, r = local row 0..11).

PE quadrant rule (operand base partitions must be equal and in {0,32,64}):
the column-DFT pass (pass1) is computed once per noise column (shared by
all windows of a row-group); the sliding window offset c enters pass2 via
row-shifted stationary matrices (one per c%32) against fixed base-0
A-segments (cols 0..95 / 32..127 / 64..159).
"""

import os
import sys

import numpy as np

sys.path.insert(0, "/opt/trn_rl_repo")

import ml_dtypes

import concourse.bass as bass
import concourse.tile as tile
from concourse import bacc, mybir
from concourse.bass_utils import run_bass_kernel_spmd

N = 65
D = 32
H = W = 96
A_, B_ = 15.0, 3.0
NR = 12           # row-groups per core
NC_ = 96          # window columns
LCORE = NR * NC_  # 1152 windows per core
NSUP = 9          # supertiles of 128 windows
KK = N * N        # 4225
MHARM = 20
F = 1 + 2 * MHARM  # 41 theta features
AW = 3 * N + 1     # 196: per-r A block [A1T | A2T | -A2T | colsum]
KH = 33            # half-plane: k2 in 0..32, weights folded into the maps
KKH = KH * N       # 2145
BF16 = mybir.dt.bfloat16
F32 = mybir.dt.float32
AF = mybir.ActivationFunctionType
OP = mybir.AluOpType

_cache = {}


def _consts():
    if "c" in _cache:
        return _cache["c"]
    k = np.arange(N)
    n_ = np.arange(N)
    # W-path DFT with center phase baked: F~[k,n] = exp(-2i pi k (n-D)/N)
    ang_w = -2 * np.pi * np.outer(k, (n_ - D)) / N
    Cw = np.cos(ang_w)
    Sw = -np.sin(ang_w)  # F~ = Cw - i*Sw
    # R-path centered DFT: Fc[k,m] = exp(-2i pi k (m+33)/N), Rfc real/even
    ang_r = -2 * np.pi * np.outer(k, (n_ + 33)) / N
    Cr = np.cos(ang_r)
    Sr = -np.sin(ang_r)

    bf = lambda x: np.ascontiguousarray(x, dtype=np.float64).astype(ml_dtypes.bfloat16)

    # pass1 rhs [65, 196]: [CwT | SwT | -SwT | ones]
    dftP1 = np.concatenate([Cw.T, Sw.T, -Sw.T, np.ones((N, 1))], axis=1)

    # pass2 shifted stationaries (k2 <= 32 columns only): per rem, rows
    # rem..rem+64 hold CwT / SwT
    shcs = np.zeros((96, 32 * 2 * KH))
    shones = np.zeros((96, 32))
    for rem in range(32):
        shcs[rem:rem + N, rem * 2 * KH:rem * 2 * KH + KH] = Cw.T[:, :KH]
        shcs[rem:rem + N, rem * 2 * KH + KH:(rem + 1) * 2 * KH] = Sw.T[:, :KH]
        shones[rem:rem + N, rem] = 1.0

    # branch-phase maps, flattened k2-major: flat[k2*65+k1] = map[k1, k2]
    K1, K2 = np.meshgrid(k, k, indexing="ij")
    phi = 2 * np.pi * ((K1 + K2) % N) / N
    psi_pos = np.where(phi > np.pi, phi - 2 * np.pi, phi)
    phin = (phi + np.pi) % (2 * np.pi)
    psi_neg = np.where(phin > np.pi, phin - 2 * np.pi, phin)
    negk1 = (-K1) % N
    negk2 = (-K2) % N
    cnn = np.cos((psi_neg + psi_neg[negk1, negk2]) / 2)
    # half-plane weights: k2=0 row once, k2=1..32 doubled (mirror half);
    # sn zeroed on the k1+k2=65 anti-diagonal (those pairs cancel exactly)
    wgt = np.where(K2 == 0, 1.0, 2.0)
    anti = (K1 + K2) == N
    flat = lambda m: m[:, :KH].T.reshape(-1)  # flat[k2*65+k1] = m[k1,k2]
    cp = flat(np.cos(psi_pos / 2) * wgt)
    sp = flat(np.sin(psi_pos / 2) * wgt)
    cn = flat(np.cos(psi_neg / 2) * wgt)
    sn = flat(np.sin(psi_neg / 2) * wgt * np.where(anti, 0.0, 1.0))
    an = flat((1 + cnn) / 2 * wgt)
    wm = flat(wgt + 0 * K1)

    # theta-Fourier basis of Rfc (period pi), NG-point grid, MHARM harmonics
    NG = 128
    x = np.linspace(-D, D, N)
    Xm, Ym = np.meshgrid(x, x, indexing="ij")
    X2, Y2, XY = Xm * Xm, Ym * Ym, Xm * Ym
    thg = np.arange(NG) * np.pi / NG
    c, s = np.cos(thg)[:, None, None], np.sin(thg)[:, None, None]
    al = (c * c) / A_**2 + (s * s) / B_**2
    be = (s * s) / A_**2 + (c * c) / B_**2
    ga = 2 * c * s * (1 / A_**2 - 1 / B_**2)
    q = al * X2[None] + be * Y2[None] + ga * XY[None]
    R = np.exp(-np.sqrt(q))  # [NG, 65, 65]
    T1 = np.einsum("ki,gij->gkj", Cr, R)
    T2 = np.einsum("ki,gij->gkj", Sr, R)
    Rfc = np.einsum("gkj,lj->gkl", T1, Cr) - np.einsum("gkj,lj->gkl", T2, Sr)
    Fh = np.fft.rfft(Rfc, axis=0) / NG
    flatB = lambda m: m[:, :KH].T.reshape(-1)
    Bas = np.empty((F, KKH))
    Bas[0] = flatB(Fh[0].real)
    for m in range(1, MHARM + 1):
        Bas[2 * m - 1] = flatB(2 * Fh[m].real)
        Bas[2 * m] = flatB(-2 * Fh[m].imag)

    out = {
        "dftP1": bf(dftP1),
        "shcs": bf(shcs),
        "shones": bf(shones),
        "idt1": np.ones((1, 1), np.float32),
        "B": bf(Bas),
        "cp": bf(np.broadcast_to(cp, (128, KKH))),
        "sp": bf(np.broadcast_to(sp, (128, KKH))),
        "cn": bf(np.broadcast_to(cn, (128, KKH))),
        "sn": bf(np.broadcast_to(sn, (128, KKH))),
        "an": bf(np.broadcast_to(an, (128, KKH))),
        "wm": bf(np.broadcast_to(wm, (128, KKH))),
    }
    _cache["c"] = out
    return out


CONST_NAMES = ["dftP1", "shcs", "shones", "idt1", "B", "cp", "sp", "cn", "sn",
               "an", "wm"]


def _build_program():
    if "nc" in _cache:
        return _cache["nc"]
    cst = _consts()
    nc = bacc.Bacc()

    inp = {}
    inp["noise"] = nc.dram_tensor("noise", [NR + N - 1, 160], BF16, kind="ExternalInput")
    inp["feat"] = nc.dram_tensor("feat", [F, LCORE], BF16, kind="ExternalInput")
    for nm in CONST_NAMES:
        dt = F32 if cst[nm].dtype == np.float32 else BF16
        inp[nm] = nc.dram_tensor(nm, list(cst[nm].shape), dt, kind="ExternalInput")
    out_d = nc.dram_tensor("out", [128, NSUP], F32, kind="ExternalOutput")

    N2 = float(KK)
    va = 1.0 / (N2 * (N2 - 1.0))
    vb = N2 / (N2 - 1.0)

    with tile.TileContext(nc) as tc:
        with (
            tc.tile_pool(name="const", bufs=1) as cpool,
            tc.tile_pool(name="abuf", bufs=1) as apool,
            tc.tile_pool(name="maps", bufs=1) as mpool,
            tc.tile_pool(name="stage", bufs=3) as spool,
            tc.tile_pool(name="work", bufs=1) as wk,
            tc.tile_pool(name="uv", bufs=2) as uvp,
            tc.tile_pool(name="wtiles", bufs=3) as wpool,
            tc.tile_pool(name="acc", bufs=2) as acc,
            tc.tile_pool(name="epi", bufs=4) as epi,
            tc.tile_pool(name="dram", bufs=1, space="DRAM") as dpool,
        ):
            # ---- load constants / inputs into SBUF ----
            sb = {}
            for nm in CONST_NAMES:
                dt = F32 if cst[nm].dtype == np.float32 else BF16
                pool = (mpool if nm in ("cp", "sp", "cn", "sn", "an", "wm", "B")
                        else cpool)
                t = pool.tile(list(cst[nm].shape), dt, tag=nm)
                nc.sync.dma_start(t[:], inp[nm][:])
                sb[nm] = t
            noise_r = []
            for r in range(NR):
                t = cpool.tile([N, 160], BF16, tag=f"nr{r}")
                nc.sync.dma_start(t[:], inp["noise"][r:r + N, :])
                noise_r.append(t)
            feat_sb = cpool.tile([F, LCORE], BF16, tag="feat")
            nc.sync.dma_start(feat_sb[:], inp["feat"][:])

            A_lo = apool.tile([128, NR * AW], BF16, tag="alo")
            A_mid = apool.tile([96, NR * AW], BF16, tag="amid")
            A_hi = apool.tile([96, NR * AW], BF16, tag="ahi")
            sw_all = apool.tile([1, LCORE], F32, tag="sw")
            sw_t = apool.tile([128, NSUP], F32, tag="swt")
            out_t = apool.tile([128, NSUP], F32, tag="outt")
            kstage = int(os.environ.get("KSTAGE", "6"))
            if kstage < 6:
                nc.vector.memset(out_t[:], 0.0)

            # ---- phase A: pass1 (column DFT of the noise slice) ----
            with tc.tile_pool(name="psA", bufs=2, space="PSUM") as psA:
                for r in range(NR):
                    pa = psA.tile([128, AW], F32, tag="pa")
                    nc.tensor.matmul(pa[:], noise_r[r][:, 0:128], sb["dftP1"][:],
                                     start=True, stop=True)
                    nc.scalar.copy(A_lo[:, r * AW:(r + 1) * AW], pa[:])
                    pb = psA.tile([96, AW], F32, tag="pb")
                    nc.tensor.matmul(pb[:], noise_r[r][:, 64:160], sb["dftP1"][:],
                                     start=True, stop=True)
                    nc.scalar.copy(A_hi[:, r * AW:(r + 1) * AW], pb[:])
                # A_mid = A_lo rows 32..127.  SBUF->SBUF direct DMAs only
                # support one sync wait, so bounce the shift through DRAM.
                dmid = dpool.tile([96, NR * AW], BF16, tag="dmid", name="dmid")
                for r in range(NR):
                    nc.sync.dma_start(dmid[:, r * AW:(r + 1) * AW],
                                      A_lo[32:128, r * AW:(r + 1) * AW])
                    nc.sync.dma_start(A_mid[:, r * AW:(r + 1) * AW],
                                      dmid[:, r * AW:(r + 1) * AW])

                seg3 = {}
                for nm, seg in (("lo", A_lo), ("mid", A_mid), ("hi", A_hi)):
                    seg3[nm] = seg[0:96, :].rearrange("p (r k) -> p r k", k=AW)

                # window sums: sw[12c+r] = sum_j colsum[r, c+j]
                for t3 in range(3):
                    c0 = t3 * 32
                    csp = psA.tile([1, 512], F32, tag="cs")
                    for c in range(c0, c0 + 32):
                        sg = seg3["lo" if c < 32 else ("mid" if c < 64 else "hi")]
                        rem = c % 32
                        nc.tensor.matmul(csp[:, (c - c0) * 12:(c - c0) * 12 + 12],
                                         sb["shones"][:, rem:rem + 1],
                                         sg[:, :, AW - 1],
                                         start=True, stop=True)
                    nc.scalar.copy(sw_all[:, c0 * 12:(c0 + 32) * 12], csp[:, 0:384])

                # transpose sw to per-supertile partition layout [128, 9]
                for s in range(NSUP):
                    pt = psA.tile([128, 1], F32, tag="swp")
                    nc.tensor.transpose(pt[:], sw_all[:, s * 128:(s + 1) * 128],
                                        sb["idt1"][:])
                    nc.scalar.copy(sw_t[:, s:s + 1], pt[:])

            dWr = [dpool.tile([128, KKH], BF16, tag=f"dwr{s}", name=f"dwr{s}")
                   for s in range(NSUP)]
            dWi = [dpool.tile([128, KKH], BF16, tag=f"dwi{s}", name=f"dwi{s}")
                   for s in range(NSUP)]

            # ---- phase B: pass2 (row DFT per window) + basis + elementwise ----
            with (
                tc.tile_pool(name="psP2", bufs=3, space="PSUM") as ps2,
                tc.tile_pool(name="psB", bufs=2, space="PSUM") as psb,
            ):
                for c in range(NC_ if kstage >= 2 else 0):
                    sg = seg3["lo" if c < 32 else ("mid" if c < 64 else "hi")]
                    rem = c % 32
                    shC = sb["shcs"][:, rem * 2 * KH:rem * 2 * KH + KH]
                    shS = sb["shcs"][:, rem * 2 * KH + KH:(rem + 1) * 2 * KH]
                    # A blocks: A1T = [0:65], A2T = [65:130], -A2T = [130:195]
                    wr = ps2.tile([KH, 2, 512], F32, tag="p2")
                    wi = ps2.tile([KH, 2, 512], F32, tag="p2")
                    # Wr^T = Cw A1T - Sw A2T ; Wi'^T = Sw A1T + Cw A2T
                    nc.tensor.matmul(wr[:, 0, 0:455], shC, sg[:, 0:7, 0:65],
                                     start=True, stop=False, skip_group_check=True)
                    nc.tensor.matmul(wr[:, 1, 0:325], shC, sg[:, 7:12, 0:65],
                                     start=True, stop=False, skip_group_check=True)
                    nc.tensor.matmul(wi[:, 0, 0:455], shC, sg[:, 0:7, 65:130],
                                     start=True, stop=False, skip_group_check=True)
                    nc.tensor.matmul(wi[:, 1, 0:325], shC, sg[:, 7:12, 65:130],
                                     start=True, stop=False, skip_group_check=True)
                    nc.tensor.matmul(wr[:, 0, 0:455], shS, sg[:, 0:7, 130:195],
                                     start=False, stop=True, skip_group_check=True)
                    nc.tensor.matmul(wr[:, 1, 0:325], shS, sg[:, 7:12, 130:195],
                                     start=False, stop=True, skip_group_check=True)
                    nc.tensor.matmul(wi[:, 0, 0:455], shS, sg[:, 0:7, 0:65],
                                     start=False, stop=True, skip_group_check=True)
                    nc.tensor.matmul(wi[:, 1, 0:325], shS, sg[:, 7:12, 0:65],
                                     start=False, stop=True, skip_group_check=True)

                    wrs = spool.tile([KH, NR * N], BF16, tag="wrs")
                    wis = spool.tile([KH, NR * N], BF16, tag="wis")
                    nc.vector.tensor_copy(wrs[:, 0:455], wr[:, 0, 0:455])
                    nc.vector.tensor_copy(wrs[:, 455:780], wr[:, 1, 0:325])
                    nc.scalar.copy(wis[:, 0:455], wi[:, 0, 0:455])
                    nc.scalar.copy(wis[:, 455:780], wi[:, 1, 0:325])

                    # scatter to window-major DRAM supertiles
                    b0 = 12 * c
                    s0, p0 = b0 // 128, b0 % 128
                    n1 = min(12, 128 - p0)
                    sr3 = wrs.rearrange("p (r k) -> p r k", k=N)
                    si3 = wis.rearrange("p (r k) -> p r k", k=N)
                    d3r = dWr[s0][p0:p0 + n1, :].rearrange("b (q k) -> q b k", k=N)
                    d3i = dWi[s0][p0:p0 + n1, :].rearrange("b (q k) -> q b k", k=N)
                    nc.sync.dma_start(d3r, sr3[:, 0:n1, :])
                    nc.sync.dma_start(d3i, si3[:, 0:n1, :])
                    if n1 < 12:
                        d3r = dWr[s0 + 1][0:12 - n1, :].rearrange("b (q k) -> q b k", k=N)
                        d3i = dWi[s0 + 1][0:12 - n1, :].rearrange("b (q k) -> q b k", k=N)
                        nc.sync.dma_start(d3r, sr3[:, n1:12, :])
                        nc.sync.dma_start(d3i, si3[:, n1:12, :])

                # ---- per-supertile elementwise ----
                for s in range(NSUP if kstage >= 3 else 0):  # noqa: B007
                    wrb = wpool.tile([128, KKH], BF16, tag="wrb")
                    wib = wpool.tile([128, KKH], BF16, tag="wib")
                    nc.sync.dma_start(wrb[:], dWr[s][:])
                    nc.sync.dma_start(wib[:], dWi[s][:])

                    u = uvp.tile([128, KKH], BF16, tag="u")
                    un = uvp.tile([128, KKH], BF16, tag="un")
                    g00 = epi.tile([128, 1], F32, tag="g00")
                    for ch in range(5):
                        wd = 512 if ch < 4 else KKH - 4 * 512
                        bp = psb.tile([128, 512], F32, tag="bp")
                        nc.tensor.matmul(bp[:, 0:wd],
                                         feat_sb[:, s * 128:(s + 1) * 128],
                                         sb["B"][:, ch * 512:ch * 512 + wd],
                                         start=True, stop=True)
                        sl = slice(ch * 512, ch * 512 + wd)
                        nc.scalar.activation(u[:, sl], bp[:, 0:wd], AF.Relu)
                        nc.scalar.activation(un[:, sl], bp[:, 0:wd], AF.Relu, scale=-1.0)
                        if ch == 0:
                            nc.scalar.activation(g00[:], bp[:, 0:1], AF.Sqrt)

                    if kstage < 4:
                        continue
                    h = wk.tile([128, KKH], BF16, tag="h")
                    g = wk.tile([128, KKH], BF16, tag="g")
                    nc.scalar.sqrt(h[:], u[:])
                    nc.scalar.sqrt(g[:], un[:])

                    t1 = wk.tile([128, KKH], BF16, tag="t1")
                    Gr = wk.tile([128, KKH], BF16, tag="Gr")
                    Gi = wk.tile([128, KKH], BF16, tag="Gi")
                    Mm = wk.tile([128, KKH], BF16, tag="Mm")
                    nc.vector.tensor_tensor(t1[:], h[:], sb["cp"][:], op=OP.mult)
                    nc.vector.tensor_tensor(Mm[:], g[:], sb["cn"][:], op=OP.mult)
                    nc.vector.tensor_tensor(Gr[:], t1[:], Mm[:], op=OP.add)
                    nc.vector.tensor_tensor(t1[:], h[:], sb["sp"][:], op=OP.mult)
                    nc.vector.tensor_tensor(Mm[:], g[:], sb["sn"][:], op=OP.mult)
                    nc.vector.tensor_tensor(Gi[:], t1[:], Mm[:], op=OP.add)
                    nc.vector.tensor_tensor(t1[:], un[:], sb["an"][:], op=OP.mult)
                    nc.vector.tensor_tensor(Mm[:], u[:], sb["wm"][:], op=OP.mult)
                    nc.vector.tensor_tensor(Mm[:], t1[:], Mm[:], op=OP.add)

                    if kstage < 5:
                        continue
                    wr2 = wk.tile([128, KKH], BF16, tag="h")
                    wi2 = wk.tile([128, KKH], BF16, tag="g")
                    nc.scalar.square(wr2[:], wrb[:])
                    nc.scalar.square(wi2[:], wib[:])

                    if os.environ.get("KTTR", "1") == "0":
                        continue
                    # tensor_tensor_reduce wedges the device in this runtime;
                    # use mult + reduce pairs instead
                    r1 = acc.tile([128, 1], F32, tag="r1")
                    r2 = acc.tile([128, 1], F32, tag="r2")
                    r3 = acc.tile([128, 1], F32, tag="r3")
                    r4 = acc.tile([128, 1], F32, tag="r4")
                    s1 = acc.tile([128, 1], F32, tag="s1")
                    nm = acc.tile([128, 1], F32, tag="nm")
                    scr = wk.tile([128, KKH], BF16, tag="t1")
                    nc.vector.tensor_tensor(scr[:], wr2[:], Mm[:], op=OP.mult)
                    nc.vector.tensor_reduce(r1[:], scr[:], mybir.AxisListType.X, OP.add)
                    scr = wk.tile([128, KKH], BF16, tag="t1")
                    nc.vector.tensor_tensor(scr[:], wi2[:], Mm[:], op=OP.mult)
                    nc.vector.tensor_reduce(r2[:], scr[:], mybir.AxisListType.X, OP.add)
                    scr = wk.tile([128, KKH], BF16, tag="t1")
                    nc.vector.tensor_tensor(scr[:], wrb[:], Gr[:], op=OP.mult)
                    nc.vector.tensor_reduce(r3[:], scr[:], mybir.AxisListType.X, OP.add)
                    scr = wk.tile([128, KKH], BF16, tag="t1")
                    nc.vector.tensor_tensor(scr[:], wib[:], Gi[:], op=OP.mult)
                    nc.vector.tensor_reduce(r4[:], scr[:], mybir.AxisListType.X, OP.add)
                    nc.vector.tensor_tensor(s1[:], r1[:], r2[:], op=OP.add)
                    nc.vector.tensor_tensor(nm[:], r3[:], r4[:], op=OP.add)

                    if kstage < 6:
                        continue
                    # epilogue: out = (num/N2 - mean) / (std + 1e-6)
                    mean = epi.tile([128, 1], F32, tag="mean")
                    tv = epi.tile([128, 1], F32, tag="tv")
                    m2 = epi.tile([128, 1], F32, tag="m2")
                    var = epi.tile([128, 1], F32, tag="var")
                    sd = epi.tile([128, 1], F32, tag="sd")
                    rs = epi.tile([128, 1], F32, tag="rs")
                    vc = epi.tile([128, 1], F32, tag="vc")
                    df = epi.tile([128, 1], F32, tag="df")
                    nc.vector.tensor_scalar(mean[:], sw_t[:, s:s + 1], g00[:],
                                            1.0 / N2, OP.mult, OP.mult)
                    nc.vector.tensor_scalar(tv[:], s1[:], va, None, OP.mult)
                    nc.vector.tensor_scalar(m2[:], mean[:], mean[:], vb,
                                            OP.mult, OP.mult)
                    nc.vector.tensor_tensor(var[:], tv[:], m2[:], op=OP.subtract)
                    nc.vector.tensor_scalar(var[:], var[:], 0.0, None, OP.max)
                    nc.scalar.sqrt(sd[:], var[:])
                    nc.vector.tensor_scalar(sd[:], sd[:], 1e-6, None, OP.add)
                    nc.vector.reciprocal(rs[:], sd[:])
                    nc.vector.tensor_scalar(vc[:], nm[:], 1.0 / N2, None, OP.mult)
                    nc.vector.tensor_tensor(df[:], vc[:], mean[:], op=OP.subtract)
                    nc.vector.tensor_tensor(out_t[:, s:s + 1], df[:], rs[:], op=OP.mult)

            # aggregate the 9 epilogue writes through one same-engine copy so
            # the output DMA has a single producer
            out_t2 = apool.tile([128, NSUP], F32, tag="outt2")
            nc.vector.tensor_copy(out_t2[:], out_t[:])
            nc.sync.dma_start(out_d[:], out_t2[:])

    nc.compile()
    _cache["nc"] = nc
    return nc


def kernel(angle_matrix, noise):
    cst = _consts()
    nc = _build_program()

    angle = np.asarray(angle_matrix, dtype=np.float64)
    noise2d = np.asarray(noise, dtype=np.float32)[0, 0]

    m = np.arange(1, MHARM + 1)
    in_maps = []
    for core in range(8):
        nslice = noise2d[12 * core:12 * core + NR + N - 1, :].astype(ml_dtypes.bfloat16)
        th = angle[12 * core:12 * core + NR, :]          # [12, 96] (r, c)
        thb = th.T.reshape(-1)                           # b = 12c + r
        feat = np.empty((F, LCORE))
        feat[0] = 1.0
        feat[1::2] = np.cos(2 * m[:, None] * thb[None, :])
        feat[2::2] = np.sin(2 * m[:, None] * thb[None, :])
        im = {"noise": nslice, "feat": feat.astype(ml_dtypes.bfloat16)}
        for nm in CONST_NAMES:
            im[nm] = cst[nm]
        in_maps.append(im)

    res = run_bass_kernel_spmd(nc, in_maps, core_ids=list(range(8)))

    out = np.empty((H, W), dtype=np.float32)
    for core in range(8):
        o = np.asarray(res.results[core]["out"])         # [128, 9]
        b = o.T.reshape(-1)                              # [1152], b = 12c + r
        out[12 * core:12 * core + 12, :] = b.reshape(NC_, NR).T
    return out
